# revision 23
# baseline (speedup 1.0000x reference)
"""BitLinear v5: fully-resident operands + readiness-ordered (c,n) jobs.

Data-parallel over batch (2048 tokens/core, full weight replicated).

Structure:
 - All 16 transposed x_q token tiles AND all 4 transposed weight n-groups
   stay resident in SBUF (64+64 KiB/partition). Matmul (token, n-group)
   jobs are emitted in estimated-readiness order, so PE consumes whatever
   is available and stays backlogged once ~2 token tiles have landed.
 - Ternary weight quant as two independent pre-scaled compares
     ac = (w >= +0.5(g+eps)) * g/QB   [Pool]
     tn = (w <= -0.5(g+eps)) * g/QB   [Pool]
     wts = ac - tn (in place)         [DVE, bf16 2x]
   so no serial compare->compare->scale chain; gamma/QB is folded into
   the bf16 weights and dequant is one DVE pass: out = ps*(m*rrms)+bias.
 - x_q = bf16(x * 127/max|x|) with no integer rounding (l2 err ~8e-3 vs
   reference, gate 2e-2). RMS enters only through the per-token output
   scale m*rrms.
 - Engine split: ACT: x square(+accum), x quant, tiny sqrts, w-transpose
   issue; DVE: w abs-sum, x absmax, wts combine, tiny stats, out dequant;
   Pool: the two w compares, w DMA issue; SP: x/out DMA + xq transposes.
"""

import sys

for _p in ("/opt/trn_rl_repo", "/opt/pypackages"):
    if _p not in sys.path:
        sys.path.append(_p)

import numpy as np

import concourse.bass as bass
import concourse.bacc as bacc
import concourse.tile as tile
from concourse import mybir
from concourse.bass_utils import run_bass_kernel_spmd

P = 128
EPS = 1e-8
QB = 127.0
F32 = mybir.dt.float32
BF16 = mybir.dt.bfloat16
AF = mybir.ActivationFunctionType
OP = mybir.AluOpType
NFREE = 512  # matmul moving free dim / PSUM bank

# estimated readiness (us) of token tile j and w tile d, for emission ordering
# (w prioritized: each w tile gates 16 jobs, an x tile only 4)
S_X, R_X, S_W, R_W = 13.0, 9.5, 2.0, 8.5  # R_W for d>=4; first 4 back-to-back


def t_w_tile(d):
    return S_W + 3.3 * min(d, 3) + R_W * max(d - 3, 0)


def t_x_tile(j):
    # x0 squeezed right behind the 4-tile w prefix; the rest paced by R_X
    return 8.5 if j == 0 else S_X + R_X * j


def _bcast_row(ap_1d, parts):
    """Broadcast a 1-D AP across `parts` partitions via a 0-stride dim."""
    return bass.AP(
        tensor=ap_1d.tensor, offset=ap_1d.offset, ap=[[0, parts]] + list(ap_1d.ap)
    )


def build_bitlinear(tc, x_d, w_d, b_d, out_d, T, D, N):
    """Emit the kernel for one core: x[T,D] fp32, w[N,D], b[N] -> out[T,N]."""
    from contextlib import ExitStack

    nc = tc.nc
    KT = D // P  # contraction tiles
    DT = N // P  # dout row tiles
    TT = T // P  # token tiles
    NT = N // NFREE  # matmul n-group tiles
    GW = DT // NT  # weight row-tiles per n-group

    with ExitStack() as ctx:
        const = ctx.enter_context(tc.tile_pool(name="const", bufs=1))
        wq = ctx.enter_context(tc.tile_pool(name="wq", bufs=4))
        acp = ctx.enter_context(tc.tile_pool(name="acp", bufs=2))
        tnp = ctx.enter_context(tc.tile_pool(name="tnp", bufs=2))
        wtT_p = ctx.enter_context(tc.tile_pool(name="wtT_p", bufs=1))
        xin = ctx.enter_context(tc.tile_pool(name="xin", bufs=3))
        xscr = ctx.enter_context(tc.tile_pool(name="xscr", bufs=2))
        xqT_p = ctx.enter_context(tc.tile_pool(name="xqT_p", bufs=1))
        ost = ctx.enter_context(tc.tile_pool(name="ost", bufs=2))
        stat = ctx.enter_context(tc.tile_pool(name="stat", bufs=3))
        psum = ctx.enter_context(tc.tile_pool(name="psum", bufs=7, space="PSUM"))
        psum_d = ctx.enter_context(tc.tile_pool(name="psum_d", bufs=1, space="PSUM"))

        # ---------------- constants ----------------
        eps_c = const.tile([P, 1], F32)
        nc.vector.memset(eps_c, 1e-8)
        zero_c = const.tile([P, 1], F32)
        nc.vector.memset(zero_c, 0.0)

        ham_ps = psum_d.tile([1, 1], F32)

        def ham_warm(col_ap):
            # 1x1 fp32 matmul reading a just-produced [P,1] column: keeps the
            # PE HAM clock-gate warm through the prologue at ~zero cost.
            nc.tensor.matmul(ham_ps[:, :], lhsT=col_ap, rhs=col_ap)

        # bias broadcast across partitions (bf16: only feeds the +bias add);
        # deferred event keeps it off the critical DMA front
        biasB = const.tile([P, N], BF16, name="biasB")

        def load_bias():
            nc.gpsimd.dma_start(out=biasB, in_=_bcast_row(b_d, P))

        gssw = const.tile([P, DT], F32)  # sum(|w|) per dout row
        thr_p = const.tile([P, DT], F32)  # +0.5*(gamma+EPS)
        thr_n = const.tile([P, DT], F32)  # -0.5*(gamma+EPS)
        gsc = const.tile([P, DT], F32)  # +gamma/QB

        def w_stages(d, wtT_tile):
            """Return [(dt_us, closure)] stages for weight tile d."""
            st = {}
            ds_ = slice(d, d + 1)

            def s_load():
                # fp32: bf16 weights flip ~1 ternary threshold decision per
                # row (w near +-gamma/2), which alone costs ~1.7e-2 rel err.
                # Alternate issue queue (Pool-SWDGE / SP): w loads are the
                # heaviest DMA stream and are dependency-free, so two issue
                # queues feed the 16 SDMA engines with no HOL risk.
                st["w"] = wq.tile([P, D], F32, name="w_tile")
                eng = nc.gpsimd if d % 2 == 0 else nc.sync
                eng.dma_start(
                    out=st["w"], in_=w_d[d * P : (d + 1) * P, :]
                )

            def s_reduce():
                nc.vector.tensor_reduce(
                    out=gssw[:, ds_],
                    in_=st["w"],
                    axis=mybir.AxisListType.X,
                    op=OP.add,
                    apply_absolute_value=True,
                )

            def s_thr():
                nc.vector.tensor_scalar(
                    out=thr_p[:, ds_], in0=gssw[:, ds_], scalar1=0.5 / D,
                    scalar2=0.5 * EPS, op0=OP.mult, op1=OP.add,
                )
                nc.vector.tensor_scalar(
                    out=thr_n[:, ds_], in0=gssw[:, ds_], scalar1=-0.5 / D,
                    scalar2=-0.5 * EPS, op0=OP.mult, op1=OP.add,
                )
                nc.vector.tensor_scalar(
                    out=gsc[:, ds_], in0=gssw[:, ds_], scalar1=0.5 / D,
                    scalar2=None, op0=OP.mult,
                )
                if d < 3:
                    ham_warm(gssw[:, ds_])
                    ham_warm(thr_p[:, ds_])
                    ham_warm(gsc[:, ds_])

            def s_cmp():
                # ternary via ACT Sign with per-partition threshold biases:
                # sign(w - g/2) + sign(w + g/2) = 2*w_t  (w_t in {-1,0,1})
                st["ac"] = acp.tile([P, D], BF16, name="ac")
                nc.scalar.activation(
                    out=st["ac"], in_=st["w"], func=AF.Sign,
                    bias=thr_n[:, ds_],
                )
                st["tn"] = tnp.tile([P, D], BF16, name="tn")
                nc.scalar.activation(
                    out=st["tn"], in_=st["w"], func=AF.Sign,
                    bias=thr_p[:, ds_],
                )

            def s_comb():
                # ac <- (ac + tn) * gamma/2 = w_t * gamma (bf16 DVE 2x)
                nc.vector.tensor_tensor(
                    out=st["ac"], in0=st["ac"], in1=st["tn"], op=OP.add
                )
                nc.vector.tensor_scalar(
                    out=st["ac"], in0=st["ac"], scalar1=gsc[:, ds_],
                    scalar2=None, op0=OP.mult,
                )

            def s_xpose():
                nc.sync.dma_start_transpose(
                    out=wtT_tile[:, :, (d % GW) * P : (d % GW + 1) * P],
                    in_=st["ac"][:, :],
                )

            return [
                (0.0, s_load), (3.4, s_reduce), (5.7, s_thr),
                (5.8, s_cmp), (10.2, s_comb), (11.4, s_xpose),
            ]

        def x_stages(j, xqT_tile, xs_out):
            """Return [(dt_us, closure)] stages for token tile j.

            x_q IS the loaded bf16 x: bf16 is scale-invariant, so the
            reference's per-token 127/max|x| quant scale cancels exactly
            against the dequant and is dropped; rms-normalization is applied
            on the output side (xs = 1/rms). No absmax, no quant pass.
            """
            st = {}

            def s_load():
                # issue from SP (w loads issue from Pool): two issue queues
                # keep more DMAs pending so the engines stay back-to-back
                st["x"] = xin.tile([P, D], BF16, name="x_tile")
                nc.sync.dma_start(
                    out=st["x"], in_=x_d[j * P : (j + 1) * P, :]
                )

            def s_xpose():
                # NOTE: issuing these from ACT (even split by parity) passes
                # the scheduler's model but CORRUPTS RESULTS on hardware
                # (rel err 5.2) -- keep on SP.
                nc.sync.dma_start_transpose(
                    out=xqT_tile[:, :, :], in_=st["x"][:, :]
                )

            def s_sq():
                sqscr = xscr.tile([P, D], BF16, name="sqscr", tag="xscr")
                st["ssc"] = stat.tile([P, 1], F32, name="ssc")
                nc.scalar.activation(
                    out=sqscr,
                    in_=st["x"],
                    func=AF.Square,
                    bias=zero_c[:, :],
                    accum_out=st["ssc"][:, :],
                )

            def s_stats():
                # xs = rrms = 1/sqrt(mean(x^2)+1e-8)
                rmsc = stat.tile([P, 1], F32, name="rmsc")
                nc.scalar.activation(
                    out=rmsc, in_=st["ssc"], func=AF.Sqrt,
                    scale=1.0 / D, bias=eps_c[:, :],
                )
                nc.vector.reciprocal(out=xs_out, in_=rmsc)
                if j < 2:
                    ham_warm(rmsc[:, :])
                    ham_warm(xs_out[:, :])

            return [
                (0.0, s_load), (1.7, s_xpose), (1.9, s_sq), (4.1, s_stats),
            ]

        # ---- globally time-ordered emission ----
        # Each engine executes its instruction stream IN ORDER, so emission
        # order IS the per-engine schedule. Estimate when each producer tile
        # and matmul job actually executes and emit everything in that order;
        # mis-ordering couples unrelated pipelines via head-of-line blocking.
        xqT = [xqT_p.tile([P, KT, P], BF16, name=f"xqT{j}") for j in range(TT)]
        wtT = [
            wtT_p.tile([P, KT, NFREE], BF16, name=f"wtTg{g}") for g in range(NT)
        ]
        xs_t = {}

        JOB_US = 3.45  # PE time per full-width (token-tile, n-group) job
        events = [(15.0, load_bias)]
        t_x_done = {}
        t_wg_done = [0.0] * NT
        t_wh0 = 0.0  # first HALF of group 0 (w tiles 0-1) transposed
        for d in range(DT):
            t0 = t_w_tile(d)
            stages = w_stages(d, wtT[d // GW])
            for dt, fn in stages:
                events.append((t0 + dt, fn))
            t_wg_done[d // GW] = max(t_wg_done[d // GW], t0 + stages[-1][0] + 1.9)
            if d == 1:
                t_wh0 = t0 + stages[-1][0] + 1.9
        for j in range(TT):
            t0 = t_x_tile(j)
            xs_t[j] = stat.tile([P, 1], F32, name="xsc", bufs=TT)
            stages = x_stages(j, xqT[j], xs_t[j])
            for dt, fn in stages:
                events.append((t0 + dt, fn))
            t_x_done[j] = t0 + stages[-1][0] + 1.9

        def emit_job_mm(n, j, st, cs):
            # cs: column sub-range of the n-group (half-width early jobs
            # only need the first 2 of the group's 4 transposed w tiles)
            ps = psum.tile([P, NFREE], F32, name="ps")
            st["ps"] = ps
            for k in range(KT):
                nc.tensor.matmul(
                    ps[:, cs[0] : cs[1]],
                    lhsT=xqT[j][:, k, :],
                    rhs=wtT[n][:, k, cs[0] : cs[1]],
                    start=(k == 0),
                    stop=(k == KT - 1),
                )

        def emit_job_out(n, j, st, cs):
            # out = psum * (m*rrms) + bias
            ns = slice(n * NFREE + cs[0], n * NFREE + cs[1])
            u = ost.tile([P, NFREE], BF16, name="u")
            nc.vector.scalar_tensor_tensor(
                out=u[:, cs[0] : cs[1]],
                in0=st["ps"][:, cs[0] : cs[1]],
                scalar=xs_t[j][:, :],
                in1=biasB[:, ns],
                op0=OP.mult,
                op1=OP.add,
            )
            nc.sync.dma_start(
                out=out_d[j * P : (j + 1) * P, ns], in_=u[:, cs[0] : cs[1]]
            )

        H = NFREE // 2
        jobs = []
        for n in range(NT):
            for j in range(TT):
                if n == 0 and j < 3:
                    # split: first half gated only on w tiles 0-1
                    jobs.append(
                        (max(t_x_done[j], t_wh0), n, j, (0, H))
                    )
                    jobs.append(
                        (max(t_x_done[j], t_wg_done[0]), n, j, (H, NFREE))
                    )
                else:
                    jobs.append(
                        (max(t_x_done[j], t_wg_done[n]), n, j, (0, NFREE))
                    )
        jobs.sort(key=lambda t: (t[0], t[1]))
        pe_t = 0.0
        for ready, n, j, cs in jobs:
            start = max(pe_t, ready)
            dur = JOB_US * (cs[1] - cs[0]) / NFREE
            pe_t = start + dur
            st = {}
            events.append(
                (start, lambda n=n, j=j, st=st, cs=cs: emit_job_mm(n, j, st, cs))
            )
            events.append(
                (start + dur + 1.0,
                 lambda n=n, j=j, st=st, cs=cs: emit_job_out(n, j, st, cs))
            )

        events.sort(key=lambda e: e[0])
        for _, fn in events:
            fn()


def build_nc(T, D, N, num_cores=8):
    nc = bacc.Bacc(
        "TRN2", target_bir_lowering=False, debug=False, num_devices=num_cores
    )
    x_d = nc.dram_tensor("x", [T, D], BF16, kind="ExternalInput")
    w_d = nc.dram_tensor("weight", [N, D], F32, kind="ExternalInput")
    b_d = nc.dram_tensor("bias", [N], F32, kind="ExternalInput")
    out_d = nc.dram_tensor("out", [T, N], BF16, kind="ExternalOutput")
    with tile.TileContext(nc) as tc:
        build_bitlinear(tc, x_d.ap(), w_d.ap(), b_d.ap(), out_d.ap(), T, D, N)
    nc.compile()
    return nc


_CACHE: dict = {}


def get_compiled(T=2048, D=2048, N=2048, num_cores=8):
    key = (T, D, N, num_cores)
    if key not in _CACHE:
        _CACHE[key] = build_nc(T, D, N, num_cores)
    return _CACHE[key]


def run(x, weight, bias, trace=False, **spmd_kwargs):
    import ml_dtypes

    bf16 = ml_dtypes.bfloat16
    x = np.ascontiguousarray(x).astype(bf16)
    weight = np.ascontiguousarray(weight, dtype=np.float32)
    bias = np.ascontiguousarray(bias, dtype=np.float32)
    B, S, D = x.shape
    N = weight.shape[0]
    num_cores = 8
    T = (B * S) // num_cores
    nc = get_compiled(T, D, N, num_cores)
    xs = x.reshape(num_cores, T, D)
    in_maps = [
        {"x": xs[c], "weight": weight, "bias": bias} for c in range(num_cores)
    ]
    res = run_bass_kernel_spmd(
        nc, in_maps, list(range(num_cores)), trace=trace, **spmd_kwargs
    )
    out = np.stack([res.results[c]["out"] for c in range(num_cores)])
    return out.reshape(B, S, N).astype(np.float32), res


def kernel(x, weight, bias):
    out, _ = run(x, weight, bias)
    return out


if __name__ == "__main__":
    rng = np.random.default_rng(0)
    x = rng.standard_normal((8, 2048, 2048), dtype=np.float32)
    w = rng.uniform(-0.05, 0.05, (2048, 2048)).astype(np.float32)
    b = (rng.standard_normal(2048) * 0.02).astype(np.float32)
    out = kernel(x, w, b)
    print(out.shape, out.dtype)
